# revision 9
# baseline (speedup 1.0000x reference)
"""Multi-head attention (16 heads, d_model=1024, head_dim=64) on 8 trn2 cores.

Sharding: core c handles batch b = c//2 and heads [8*(c%2), 8*(c%2)+8)
(data parallel over batch x tensor parallel over heads). Each core
computes its 8 heads' Q/K/V projections, attention, and a partial output
projection; the host sums the two partial projections per batch element
(the "all-reduce") and adds the output bias.

Device-side layout is feature-major ("transposed"): projections produce
Q^T/K^T [d, t] so that the attention matmuls contract along partitions.
Attention output is produced as AttnOut^T [f, t], which feeds the output
projection as the stationary operand without any transposes.

All matmul inputs are bf16 (fp32 PSUM accumulation); softmax is unnormalized
exp (no max subtraction: energies are bounded ~|15| for this problem size)
with the row-sum computed by an extra ones-column in the attn@V matmul.
Measured end-to-end relative error vs the fp32 reference: ~9e-3.
"""

import numpy as np
import ml_dtypes

from concourse import bass, bacc, tile, mybir
from concourse.bass_utils import run_bass_kernel_spmd

BF16 = ml_dtypes.bfloat16
dt = mybir.dt
AF = mybir.ActivationFunctionType

N_CORES = 8
T = 2048          # tokens per batch element
D = 1024          # model dim
FH = 512          # features (head dims) per core: 8 heads x 64
NH_LOC = 8        # heads per core
HD = 64           # head dim

_prog_cache = {}


def _build_program():
    nc = bacc.Bacc("TRN2", target_bir_lowering=False, debug=False,
                   num_devices=N_CORES)

    xT = nc.dram_tensor("xT", [D, T], dt.bfloat16, kind="ExternalInput").ap()
    wqT = nc.dram_tensor("wqT", [D, FH], dt.bfloat16, kind="ExternalInput").ap()
    wkT = nc.dram_tensor("wkT", [D, FH], dt.bfloat16, kind="ExternalInput").ap()
    wvT = nc.dram_tensor("wvT", [D, FH], dt.bfloat16, kind="ExternalInput").ap()
    bqT = nc.dram_tensor("bqT", [128, 4], dt.float32, kind="ExternalInput").ap()
    bkT = nc.dram_tensor("bkT", [128, 4], dt.float32, kind="ExternalInput").ap()
    bvs = nc.dram_tensor("bvs", [1, FH], dt.bfloat16, kind="ExternalInput").ap()
    wpT = nc.dram_tensor("wpT", [FH, D], dt.bfloat16, kind="ExternalInput").ap()
    ones = nc.dram_tensor("ones", [1, T], dt.bfloat16, kind="ExternalInput").ap()
    out = nc.dram_tensor("out", [T, D], dt.float32, kind="ExternalOutput").ap()

    with tile.TileContext(nc) as tc:
        _emit(tc, out, xT, wqT, wkT, wvT, bqT, bkT, bvs, wpT, ones)
    nc.compile()
    return nc


def _emit(tc, out, xT, wqT, wkT, wvT, bqT, bkT, bvs, wpT, ones):
    nc = tc.nc
    f32 = dt.float32
    bf16 = dt.bfloat16

    with (
        tc.tile_pool(name="wp_pool", bufs=1) as wp_pool,
        tc.tile_pool(name="qkv_sb", bufs=1) as qkv_sb,
        tc.tile_pool(name="ao_pool", bufs=1) as ao_pool,
        tc.tile_pool(name="const", bufs=1) as const,
    ):
        wp_s = wp_pool.tile([128, 4, D], bf16)
        nc.sync.dma_start(out=wp_s[:], in_=wpT.rearrange("(c p) o -> p c o", p=128))
        ones_s = const.tile([1, T], bf16)
        nc.sync.dma_start(out=ones_s[:], in_=ones)
        bvs_s = const.tile([1, FH], bf16)
        nc.sync.dma_start(out=bvs_s[:], in_=bvs)
        bqT_s = const.tile([128, 4], f32)
        nc.sync.dma_start(out=bqT_s[:], in_=bqT)
        bkT_s = const.tile([128, 4], f32)
        nc.sync.dma_start(out=bkT_s[:], in_=bkT)

        # QT/KT: [d-in-pair(128), head-pair(4), t] ; V: [t-in-chunk(128),
        # t-chunk(16), head(8), 66] with col 64 = 1.0 (row-sum trick).
        QT_sb = qkv_sb.tile([128, 4, T], bf16)
        KT_sb = qkv_sb.tile([128, 4, T], bf16)
        V_sb = qkv_sb.tile([128, 16, NH_LOC, 66], bf16)
        nc.vector.memset(V_sb[:, :, :, 64:66], 1.0)

        # AttnOut^T: [f-in-chunk(128), f-chunk(4), t]
        AO_sb = ao_pool.tile([128, 4, T], bf16)

        # ---------------- Phase 1: Q/K/V projections ----------------
        with (
            tc.tile_pool(name="x_pool", bufs=1) as x_pool,
            tc.tile_pool(name="w_in", bufs=1) as w_in,
            tc.tile_pool(name="ps_qkv", bufs=8, space="PSUM") as ps_qkv,
        ):
            x_s = x_pool.tile([128, 8, T], bf16)
            nc.sync.dma_start(out=x_s[:], in_=xT.rearrange("(m p) t -> p m t", p=128))
            wq_s = w_in.tile([128, 8, FH], bf16, tag="wq")
            nc.sync.dma_start(out=wq_s[:], in_=wqT.rearrange("(m p) d -> p m d", p=128))
            wk_s = w_in.tile([128, 8, FH], bf16, tag="wk")
            nc.sync.dma_start(out=wk_s[:], in_=wkT.rearrange("(m p) d -> p m d", p=128))
            wv_s = w_in.tile([128, 8, FH], bf16, tag="wv")
            nc.sync.dma_start(out=wv_s[:], in_=wvT.rearrange("(m p) d -> p m d", p=128))

            # Q^T and K^T: out[d, t] = sum_m W[m, d] * xT[m, t] (+ bias[d])
            for w_s, b_s, dst in ((wq_s, bqT_s, QT_sb), (wk_s, bkT_s, KT_sb)):
                for dc in range(4):
                    pss = [ps_qkv.tile([128, 512], f32, tag="ps", name=f"ps{n}") for n in range(4)]
                    for m in range(8):
                        lhsT = w_s[:, m, dc * 128:(dc + 1) * 128]
                        for n in range(4):
                            nc.tensor.matmul(
                                pss[n][:], lhsT, x_s[:, m, n * 512:(n + 1) * 512],
                                start=(m == 0), stop=(m == 7))
                    for n in range(4):
                        nc.vector.tensor_scalar_add(
                            dst[:, dc, n * 512:(n + 1) * 512], pss[n][:],
                            b_s[:, dc:dc + 1])

            # V (natural layout): out[t, d] = sum_m xT[m, t] * wvT[m, d] + bv[d]
            for t in range(16):
                ps = ps_qkv.tile([128, 512], f32, tag="ps")
                for m in range(8):
                    nc.tensor.matmul(ps[:], x_s[:, m, t * 128:(t + 1) * 128],
                                     wv_s[:, m, :], start=(m == 0), stop=False)
                nc.tensor.matmul(ps[:], ones_s[:, t * 128:(t + 1) * 128], bvs_s[:],
                                 start=False, stop=True)
                nc.vector.tensor_copy(
                    V_sb[:, t, :, 0:64],
                    ps[:].rearrange("p (h d) -> p h d", h=NH_LOC))

        # ---------------- Phase 2: attention ----------------
        with (
            tc.tile_pool(name="pb_pool", bufs=4) as pb_pool,
            tc.tile_pool(name="rr_pool", bufs=2) as rr_pool,
            tc.tile_pool(name="bc_pool", bufs=2) as bc_pool,
            tc.tile_pool(name="ps_e", bufs=2, space="PSUM") as ps_e,
            tc.tile_pool(name="ps_av", bufs=2, space="PSUM") as ps_av,
            tc.tile_pool(name="ps_bc", bufs=2, space="PSUM") as ps_bc,
        ):
            for hp in range(4):
                for j in range(4):
                    qsl = slice(j * 512, (j + 1) * 512)
                    # E^T = K^T.T @ Q^T per head; both heads of the pair run
                    # concurrently via PE row tiling (partition bases 0 / 64).
                    pb = [pb_pool.tile([128, 16, 512], bf16, tag="pb",
                                        name=f"pb{s}") for s in range(2)]
                    for g in range(8):
                        ee = [ps_e.tile([128, 2, 512], f32, tag="e",
                                         name=f"e{s}") for s in range(2)]
                        for i in range(2):
                            kc = 2 * g + i
                            ksl = slice(kc * 128, (kc + 1) * 128)
                            for s in range(2):
                                psl = slice(64 * s, 64 * s + 64)
                                nc.tensor.matmul(
                                    ee[s][:, i, :],
                                    KT_sb[psl, hp, ksl],
                                    QT_sb[psl, hp, qsl],
                                    start=True, stop=True)
                        for s in range(2):
                            nc.scalar.activation(
                                pb[s][:, 2 * g:2 * g + 2, :], ee[s][:], AF.Exp)

                    # attn@V with fused row-sums (V col 64 is ones)
                    for s in range(2):
                        h = 2 * hp + s
                        av = ps_av.tile([65, 512], f32, tag="av")
                        for kc in range(16):
                            nc.tensor.matmul(av[:], V_sb[:, kc, h, 0:65],
                                             pb[s][:, kc, :],
                                             start=(kc == 0), stop=(kc == 15))
                        # softmax normalization: recip of sums, broadcast
                        # across the 64 head-dim partitions via a K=1 matmul
                        rr = rr_pool.tile([1, 512], f32, tag="rr")
                        nc.vector.reciprocal(rr[:], av[64:65, :])
                        rrb = rr_pool.tile([1, 512], bf16, tag="rrb")
                        nc.vector.tensor_copy(rrb[:], rr[:])
                        bcp = ps_bc.tile([64, 512], f32, tag="bcp")
                        nc.tensor.matmul(bcp[:], ones_s[0:1, 0:64], rrb[:],
                                         start=True, stop=True)
                        bcs = bc_pool.tile([64, 512], f32, tag="bc")
                        nc.vector.tensor_copy(bcs[:], bcp[:])
                        nc.vector.tensor_mul(
                            AO_sb[64 * s:64 * s + 64, hp, qsl],
                            av[0:64, :], bcs[:])

        # ---------------- Phase 3: output projection (partial) ----------------
        with (
            tc.tile_pool(name="ostage", bufs=3) as ostage,
            tc.tile_pool(name="ps_pj", bufs=4, space="PSUM") as ps_pj,
        ):
            for t in range(16):
                tsl = slice(t * 128, (t + 1) * 128)
                ps0 = ps_pj.tile([128, 512], f32, tag="pj")
                ps1 = ps_pj.tile([128, 512], f32, tag="pj")
                for fc in range(4):
                    lhsT = AO_sb[:, fc, tsl]
                    nc.tensor.matmul(ps0[:], lhsT, wp_s[:, fc, 0:512],
                                     start=(fc == 0), stop=(fc == 3))
                    nc.tensor.matmul(ps1[:], lhsT, wp_s[:, fc, 512:1024],
                                     start=(fc == 0), stop=(fc == 3))
                st = ostage.tile([128, D], f32, tag="st")
                nc.scalar.copy(st[:, 0:512], ps0[:])
                nc.scalar.copy(st[:, 512:1024], ps1[:])
                nc.sync.dma_start(out=out[tsl, :], in_=st[:])


def get_program():
    if "nc" not in _prog_cache:
        _prog_cache["nc"] = _build_program()
    return _prog_cache["nc"]


def make_in_maps(inputs):
    x = np.asarray(inputs["x"], dtype=np.float32)
    Wq = np.asarray(inputs["Wq"], dtype=np.float32)
    bq = np.asarray(inputs["bq"], dtype=np.float32)
    Wk = np.asarray(inputs["Wk"], dtype=np.float32)
    bk = np.asarray(inputs["bk"], dtype=np.float32)
    Wv = np.asarray(inputs["Wv"], dtype=np.float32)
    bv = np.asarray(inputs["bv"], dtype=np.float32)
    Wp = np.asarray(inputs["Wp"], dtype=np.float32)

    ones_h = np.ones((1, T), dtype=BF16)
    in_maps = []
    for c in range(N_CORES):
        b, half = divmod(c, 2)
        fs = slice(half * FH, half * FH + FH)
        in_maps.append({
            "xT": np.ascontiguousarray(x[b].T).astype(BF16),
            "wqT": np.ascontiguousarray(Wq[fs].T).astype(BF16),
            "wkT": np.ascontiguousarray(Wk[fs].T).astype(BF16),
            "wvT": np.ascontiguousarray(Wv[fs].T).astype(BF16),
            "bqT": np.ascontiguousarray(bq[fs].reshape(4, 128).T),
            "bkT": np.ascontiguousarray(bk[fs].reshape(4, 128).T),
            "bvs": bv[fs].astype(BF16).reshape(1, FH),
            "wpT": np.ascontiguousarray(Wp[:, fs].T).astype(BF16),
            "ones": ones_h,
        })
    return in_maps


def gather_output(results, bp):
    bp = np.asarray(bp, dtype=np.float32)
    return np.stack([
        results[2 * b]["out"] + results[2 * b + 1]["out"] + bp[None, :]
        for b in range(4)
    ]).astype(np.float32)


def kernel(**inputs):
    nc = get_program()
    in_maps = make_in_maps(inputs)
    res = run_bass_kernel_spmd(nc, in_maps, list(range(N_CORES))).results
    return gather_output(res, inputs["bp"])
